# revision 8
# baseline (speedup 1.0000x reference)
"""Trainium2 Bass kernel for LocallyConnected1D (filters=1, k=1, no bias):

    out[b, s, 0] = sum_c x[b, s, c] * W[s, c]

x: (256, 8192, 64) f32, W: (8192, 64) f32, out: (256, 8192, 1) f32.

Strategy: data-parallel over batch across the 8 NeuronCores (32
batches/core, W replicated, no collectives).  Memory-bound: 64 MiB of x
per core must stream from HBM; the kernel sustains ~395 GB/s read-side
by striping every tile across two DMA queues.

Per core, a tile of 2 batches views as [128 partitions, 8192 free].
Each tile streams as two column slices on two queues in parallel:
  - cols [0:4416) on the sync HWDGE queue as fp32, cast to bf16 on the
    otherwise-idle ACT engine,
  - cols [4416:8192) on the SWDGE queue (nc.gpsimd) with the fp32->bf16
    cast done inside the DMA.  W's lower partition half also cast-loads
    there ahead of the x stream; the replica for the upper half is an
    SBUF->SBUF DMA on the scalar queue.
Compute runs in bf16 on DVE at full-tile granularity: in-place
tensor_mul (2x_1p mode), then the group-of-64 channel reduction as a
pairwise-add tree (bf16 TT adds also run 2x; tensor_reduce is stuck at
1x), last level into fp32.  The tree runs under high_priority so the
scheduler doesn't hoist the next tile's mul above it (in-order DVE:
that reorder turns the tail into two serial trees).  Out tiles (64 KiB)
store on the scalar queue, issued three tiles late so the ACT cast
stream never blocks on compute.  The final tile's fp32 slice loads and
casts in two chunks so the tail stays short.
"""

import sys
from contextlib import ExitStack

import numpy as np

for _p in ("/opt/trn_rl_repo", "/root/.axon_site/_ro/trn_rl_repo"):
    if _p not in sys.path:
        sys.path.insert(0, _p)

import concourse.bacc as bacc
import concourse.mybir as mybir
import concourse.tile as tile
from concourse.bass_utils import run_bass_kernel_spmd

B, S, C = 256, 8192, 64
NCORES = 8
BPC = B // NCORES          # 32 batches per core
BPT = 2                    # batches per tile
NT = BPC // BPT            # 16 tiles per core
P = 128
FREE = BPT * S * C // P    # 8192 elems per partition line
ACOLS = 4416               # fp32/ACT-cast slice (sync queue); mult of 64
JP = BPT * S // P          # 128 outputs per partition line
PREFETCH = 3
SDELAY = 3                 # out-store issue delay (tiles)

_cache = {}

BF16 = mybir.dt.bfloat16
F32 = mybir.dt.float32


def _build():
    nc = bacc.Bacc("TRN2", debug=False, target_bir_lowering=False)
    x = nc.dram_tensor("x", [BPC * S * C], F32, kind="ExternalInput").ap()
    w = nc.dram_tensor("w", [S * C], F32, kind="ExternalInput").ap()
    out = nc.dram_tensor("out", [BPC * S], F32, kind="ExternalOutput").ap()

    x_v = x.rearrange("(i p f) -> i p f", i=NT, p=P)      # [16, 128, 8192]
    w_v = w.rearrange("(p f) -> p f", p=P // 2)           # [64, 8192]
    o_v = out.rearrange("(i p j) -> i p j", i=NT, p=P)    # [16, 128, 128]

    with tile.TileContext(nc) as tc, ExitStack() as ctx:
        xp = ctx.enter_context(tc.tile_pool(name="xp", bufs=5))
        xfp = ctx.enter_context(tc.tile_pool(name="xfp", bufs=3))
        wp = ctx.enter_context(tc.tile_pool(name="wp", bufs=1))
        t1p = ctx.enter_context(tc.tile_pool(name="t1p", bufs=3))
        s2p = ctx.enter_context(tc.tile_pool(name="s2p", bufs=3))
        op = ctx.enter_context(tc.tile_pool(name="op", bufs=6))

        # W: two SWDGE cast-loads (one per replicated partition half) on
        # the gpsimd queue, ahead of the x stream.  No ACT, no sb2sb, no
        # intra-W dependencies -> ready by ~12 us, descriptor gen never
        # stalls.
        wt = wp.tile([P, FREE], BF16)
        nc.gpsimd.dma_start(wt[0 : P // 2, :], w_v[:, :])
        nc.gpsimd.dma_start(wt[P // 2 : P, :], w_v[:, :])

        xts = []
        xfs = {}

        def issue_loads(i):
            xt = xp.tile([P, FREE], BF16)
            xf = xfp.tile([P, ACOLS], F32)
            if i == NT - 1:
                # Last tile: chunk the fp32 slice so load+cast pipeline.
                hc = ACOLS // 2
                nc.sync.dma_start(xf[:, 0:hc], x_v[i][:, 0:hc])
                nc.sync.dma_start(xf[:, hc:ACOLS], x_v[i][:, hc:ACOLS])
            else:
                nc.sync.dma_start(xf[:], x_v[i][:, 0:ACOLS])
            nc.gpsimd.dma_start(xt[:, ACOLS:FREE], x_v[i][:, ACOLS:FREE])
            xts.append(xt)
            xfs[i] = xf

        for i in range(min(PREFETCH, NT)):
            issue_loads(i)

        ots = []
        for i in range(NT):
            if i + PREFETCH < NT:
                issue_loads(i + PREFETCH)
            xt = xts[i]
            xf = xfs.pop(i)
            if i == NT - 1:
                hc = ACOLS // 2
                nc.scalar.copy(xt[:, 0:hc], xf[:, 0:hc])
                nc.scalar.copy(xt[:, hc:ACOLS], xf[:, hc:ACOLS])
            else:
                nc.scalar.copy(xt[:, 0:ACOLS], xf[:])

            nc.vector.tensor_mul(xt[:], xt[:], wt[:])

            # Group-of-64 reduction: bf16 pairwise-add tree on DVE.
            with tc.high_priority():
                x3 = xt[:].rearrange("p (j c) -> p j c", c=C)      # [p,128,64]
                t1 = t1p.tile([P, JP * 32], BF16)
                t1v = t1[:].rearrange("p (j c) -> p j c", c=32)
                nc.vector.tensor_add(t1v, x3[:, :, 0:32], x3[:, :, 32:64])
                s2 = s2p.tile([P, JP * 30], BF16)
                l2 = s2[:, 0 : JP * 16].rearrange("p (j c) -> p j c", c=16)
                nc.vector.tensor_add(l2, t1v[:, :, 0:16], t1v[:, :, 16:32])
                l3 = s2[:, JP * 16 : JP * 24].rearrange("p (j c) -> p j c", c=8)
                nc.vector.tensor_add(l3, l2[:, :, 0:8], l2[:, :, 8:16])
                l4 = s2[:, JP * 24 : JP * 28].rearrange("p (j c) -> p j c", c=4)
                nc.vector.tensor_add(l4, l3[:, :, 0:4], l3[:, :, 4:8])
                l5 = s2[:, JP * 28 : JP * 30].rearrange("p (j c) -> p j c", c=2)
                nc.vector.tensor_add(l5, l4[:, :, 0:2], l4[:, :, 2:4])
                ot = op.tile([P, JP], F32)
                o3 = ot[:].rearrange("p (j c) -> p j c", c=1)
                nc.vector.tensor_add(o3, l5[:, :, 0:1], l5[:, :, 1:2])
            ots.append(ot)
            if i >= SDELAY:
                nc.scalar.dma_start(o_v[i - SDELAY], ots[i - SDELAY][:])
        for i in range(NT - SDELAY, NT):
            nc.scalar.dma_start(o_v[i], ots[i][:])

    nc.compile()
    return nc


def _get_nc():
    if "nc" not in _cache:
        _cache["nc"] = _build()
    return _cache["nc"]


def run_sharded(x, W, **spmd_kwargs):
    """Shard, run on 8 cores, gather. Returns (out[B, S], BassKernelResults)."""
    nc = _get_nc()
    xf = np.ascontiguousarray(x, dtype=np.float32).reshape(NCORES, BPC * S * C)
    wf = np.ascontiguousarray(W, dtype=np.float32).reshape(S * C)
    in_maps = [{"x": xf[i], "w": wf} for i in range(NCORES)]
    r = run_bass_kernel_spmd(nc, in_maps, list(range(NCORES)), **spmd_kwargs)
    out = np.concatenate(
        [np.asarray(r.results[i]["out"]).reshape(BPC, S) for i in range(NCORES)],
        axis=0,
    )
    return out, r


def kernel(x, W):
    out, _ = run_sharded(x, W)
    return out[..., None].astype(np.float32)
